# revision 9
# baseline (speedup 1.0000x reference)
"""Paged-attention decode kernel for Trainium2 (Bass/Tile), 8 NeuronCores.

Sharding: one KV head per core (N_KV=8). Each core gets x^T plus its head's
slices of Wq/Wk/Wv/Wo and of the paged K/V caches (K|V rows interleaved into
one [slots, 256] tensor so each gathered row is a single contiguous 1KB DMA
piece), computes its 4 query heads' attention and a partial output projection
[B, D]; the host sums the 8 partials.

Only the valid context rows (t < context_lens[b]) are streamed from the cache;
the program is specialized at trace time to the actual context_lens /
block_tables (both are needed host-side anyway to build gather patterns).
"""
import os
import sys
from contextlib import ExitStack

import numpy as np

for _p in ("/opt/trn_rl_repo", "/opt/pypackages"):
    if os.path.isdir(_p) and _p not in sys.path:
        sys.path.append(_p)

import concourse.bass as bass  # noqa: E402
import concourse.tile as tile  # noqa: E402
from concourse import bacc, mybir  # noqa: E402
from concourse.bass_utils import run_bass_kernel_spmd  # noqa: E402

N_HEADS = 32
N_KV = 8
HEAD_DIM = 128
BLOCK_SIZE = 16
MAX_SEQ = 2048
ROPE_BASE = 10000.0
SCALE = HEAD_DIM ** -0.5
B = 32
D = 4096
G = N_HEADS // N_KV  # 4 query heads per kv head
GD = G * HEAD_DIM    # 512
N_CORES = 8
MAX_CH = MAX_SEQ // 128  # 16
KVW = 2 * HEAD_DIM       # 256: one K row + one V row
CW = KVW + 1             # 257: chunk block width incl. ones column

F32 = mybir.dt.float32

LAST_RESULTS = None  # test harness reads exec_time_ns from here


def _kv_runs(bt_row, L):
    """Maximal runs of consecutive cache rows covering t in [0, L).

    Returns [(row_start, t_start, n_rows)].  With an arange block table this
    is a single run."""
    nblocks = (L + BLOCK_SIZE - 1) // BLOCK_SIZE
    runs = []
    cur_s = cur_t = cur_n = 0
    for j in range(nblocks):
        rows = min(BLOCK_SIZE, L - j * BLOCK_SIZE)
        s = int(bt_row[j]) * BLOCK_SIZE
        if cur_n and s == cur_s + cur_n:
            cur_n += rows
        else:
            if cur_n:
                runs.append((cur_s, cur_t, cur_n))
            cur_s, cur_t, cur_n = s, j * BLOCK_SIZE, rows
    if cur_n:
        runs.append((cur_s, cur_t, cur_n))
    return runs


def _emit_kv_dmas(eng, kv_d, tl, runs):
    """DMA interleaved K|V cache rows into tile `tl` laid out
    [128 (t%128), (t//128)*257 + d] with col 256 of each block left for the
    ones column."""
    for (row0, t0, n) in runs:
        while n > 0:
            c = t0 // 128
            p0 = t0 % 128
            take = min(n, 128 - p0)
            if p0 == 0 and take == 128 and n >= 128:
                nfull = n // 128
                src = kv_d[row0:row0 + nfull * 128, :].rearrange(
                    "(th tl) d -> tl th d", tl=128)
                dst = tl[:].rearrange("p (ch e) -> p ch e", e=CW)[
                    :, c:c + nfull, 0:KVW]
                eng.dma_start(dst, src)
                row0 += nfull * 128
                t0 += nfull * 128
                n -= nfull * 128
                continue
            src = kv_d[row0:row0 + take, :]
            dst = tl[p0:p0 + take, c * CW:c * CW + KVW]
            eng.dma_start(dst, src)
            row0 += take
            t0 += take
            n -= take


def _build_nc(Ls, runs_all):
    nc = bacc.Bacc("TRN2", target_bir_lowering=False, debug=False,
                   num_devices=N_CORES)

    xt_d = nc.declare_dram_parameter("xT", [D, B], F32, isOutput=False)
    wq_d = nc.declare_dram_parameter("wq", [D, GD], F32, isOutput=False)
    wk_d = nc.declare_dram_parameter("wk", [D, HEAD_DIM], F32, isOutput=False)
    wv_d = nc.declare_dram_parameter("wv", [D, HEAD_DIM], F32, isOutput=False)
    wo_d = nc.declare_dram_parameter("wo", [GD, D], F32, isOutput=False)
    kv_d = nc.declare_dram_parameter("kv", [B * MAX_SEQ, KVW], F32,
                                     isOutput=False)
    cq_d = nc.declare_dram_parameter("cq", [64, B], F32, isOutput=False)
    sq_d = nc.declare_dram_parameter("sq", [64, B], F32, isOutput=False)
    cb_d = nc.declare_dram_parameter("cb", [B, 64], F32, isOutput=False)
    sb_d = nc.declare_dram_parameter("sb", [B, 64], F32, isOutput=False)
    id_d = nc.declare_dram_parameter("ident", [128, 128], F32, isOutput=False)
    out_d = nc.declare_dram_parameter("out", [B, D], F32, isOutput=True)

    with tile.TileContext(nc) as tc, ExitStack() as top:
        cpool = top.enter_context(tc.tile_pool(name="const", bufs=1))
        ident = cpool.tile([128, 128], F32, tag="ident")
        nc.sync.dma_start(ident[:], id_d[:])
        qT = cpool.tile([128, G * B], F32, tag="qT")      # [d, g*32+b] (roped)
        knvn = cpool.tile([B, KVW], F32, tag="knvn")      # roped new k | new v
        pvn = cpool.tile([128, 128], F32, tag="pvn")      # normalized [b*4+g, d]
        pvT = cpool.tile([128, 128], F32, tag="pvT")      # [d, b*4+g]

        # ---- phase 1: q/k/v projections + rope ---------------------------
        with ExitStack() as s1:
            p1 = s1.enter_context(tc.tile_pool(name="p1", bufs=1))
            wqp = s1.enter_context(tc.tile_pool(name="wqp", bufs=4))
            ps_q = s1.enter_context(
                tc.tile_pool(name="ps_q", bufs=4, space="PSUM"))
            ps_kv = s1.enter_context(
                tc.tile_pool(name="ps_kv", bufs=2, space="PSUM"))
            tmp = s1.enter_context(tc.tile_pool(name="rtmp", bufs=4))

            xT = p1.tile([128, 32 * B], F32, tag="xT")    # [d, kc*32+b]
            nc.sync.dma_start(
                xT[:].rearrange("p (kc b) -> p kc b", b=B),
                xt_d[:].rearrange("(kc p) b -> p kc b", p=128))
            cq = p1.tile([64, B], F32, tag="cq")
            sq = p1.tile([64, B], F32, tag="sq")
            cb = p1.tile([B, 64], F32, tag="cb")
            sb = p1.tile([B, 64], F32, tag="sb")
            nc.sync.dma_start(cq[:], cq_d[:])
            nc.sync.dma_start(sq[:], sq_d[:])
            nc.sync.dma_start(cb[:], cb_d[:])
            nc.sync.dma_start(sb[:], sb_d[:])
            wk_sb = p1.tile([128, 32 * HEAD_DIM], F32, tag="wk")
            wv_sb = p1.tile([128, 32 * HEAD_DIM], F32, tag="wv")
            nc.scalar.dma_start(
                wk_sb[:].rearrange("p (kc m) -> p kc m", m=HEAD_DIM),
                wk_d[:].rearrange("(kc p) m -> p kc m", p=128))
            nc.scalar.dma_start(
                wv_sb[:].rearrange("p (kc m) -> p kc m", m=HEAD_DIM),
                wv_d[:].rearrange("(kc p) m -> p kc m", p=128))

            qg_ps = [ps_q.tile([128, B], F32, tag="ps_q", name=f"qg{g}")
                     for g in range(G)]
            k_ps = ps_kv.tile([B, HEAD_DIM], F32, tag="ps_kv")
            v_ps = ps_kv.tile([B, HEAD_DIM], F32, tag="ps_kv")
            for kc in range(32):
                wq_t = wqp.tile([128, GD], F32, tag="wq")
                eng = nc.sync if kc % 2 == 0 else nc.scalar
                eng.dma_start(wq_t[:], wq_d[kc * 128:(kc + 1) * 128, :])
                rx = xT[:, kc * B:(kc + 1) * B]
                for g in range(G):
                    nc.tensor.matmul(
                        qg_ps[g][:], wq_t[:, g * 128:(g + 1) * 128], rx,
                        start=(kc == 0), stop=(kc == 31))
                nc.tensor.matmul(k_ps[:], rx,
                                 wk_sb[:, kc * 128:(kc + 1) * 128],
                                 start=(kc == 0), stop=(kc == 31))
                nc.tensor.matmul(v_ps[:], rx,
                                 wv_sb[:, kc * 128:(kc + 1) * 128],
                                 start=(kc == 0), stop=(kc == 31))

            # rope q: rows = d within head, cols = b; per g
            for g in range(G):
                q0 = qg_ps[g][0:64, :]
                q1 = qg_ps[g][64:128, :]
                o0 = qT[0:64, g * B:(g + 1) * B]
                o1 = qT[64:128, g * B:(g + 1) * B]
                t1 = tmp.tile([64, B], F32, tag="rt1")
                t2 = tmp.tile([64, B], F32, tag="rt2")
                nc.vector.tensor_mul(t1[:], q0, cq[:])
                nc.vector.tensor_mul(t2[:], q1, sq[:])
                nc.vector.tensor_sub(o0, t1[:], t2[:])
                t3 = tmp.tile([64, B], F32, tag="rt1")
                t4 = tmp.tile([64, B], F32, tag="rt2")
                nc.vector.tensor_mul(t3[:], q0, sq[:])
                nc.vector.tensor_mul(t4[:], q1, cq[:])
                nc.vector.tensor_add(o1, t3[:], t4[:])

            # rope k (natural layout [b, d]) into knvn; v straight copy
            k0 = k_ps[:, 0:64]
            k1 = k_ps[:, 64:128]
            u1 = tmp.tile([B, 64], F32, tag="ru1")
            u2 = tmp.tile([B, 64], F32, tag="ru2")
            nc.vector.tensor_mul(u1[:], k0, cb[:])
            nc.vector.tensor_mul(u2[:], k1, sb[:])
            nc.vector.tensor_sub(knvn[:, 0:64], u1[:], u2[:])
            u3 = tmp.tile([B, 64], F32, tag="ru1")
            u4 = tmp.tile([B, 64], F32, tag="ru2")
            nc.vector.tensor_mul(u3[:], k0, sb[:])
            nc.vector.tensor_mul(u4[:], k1, cb[:])
            nc.vector.tensor_add(knvn[:, 64:128], u3[:], u4[:])
            nc.vector.tensor_copy(knvn[:, 128:256], v_ps[:])

        # ---- phase 2: per-request attention ------------------------------
        with ExitStack() as s3:
            kvpool = s3.enter_context(tc.tile_pool(name="KV", bufs=3))
            ktpool = s3.enter_context(tc.tile_pool(name="KT", bufs=20))
            scpool = s3.enter_context(tc.tile_pool(name="SC", bufs=3))
            ps_kt = s3.enter_context(
                tc.tile_pool(name="ps_kt", bufs=3, space="PSUM"))
            ps_qk = s3.enter_context(
                tc.tile_pool(name="ps_qk", bufs=2, space="PSUM"))
            ps_pv = s3.enter_context(
                tc.tile_pool(name="ps_pv", bufs=2, space="PSUM"))
            nrmpool = s3.enter_context(tc.tile_pool(name="nrm", bufs=3))

            for b in range(B):
                L = Ls[b]
                pos = L - 1
                nch = (L + 127) // 128
                eng = nc.sync if b % 2 == 0 else nc.scalar
                KVt = kvpool.tile([128, MAX_CH * CW], F32, tag="KV")
                _emit_kv_dmas(eng, kv_d, KVt, runs_all[b])
                # overwrite the new token's row (DMA: partition remap)
                prow, pch = pos % 128, pos // 128
                nc.gpsimd.dma_start(
                    KVt[prow:prow + 1, pch * CW:pch * CW + KVW],
                    knvn[b:b + 1, :])
                # ones column for the softmax denominator
                ones_ap = KVt[:].rearrange("p (ch e) -> p ch e", e=CW)[
                    :, 0:nch, KVW]
                nc.gpsimd.memset(ones_ap, 1.0)

                qk = ps_qk.tile([128, MAX_CH * G], F32, tag="ps_qk")
                sc = scpool.tile([128, MAX_CH * G], F32, tag="SC")
                rq = qT[:].rearrange("p (g b) -> p g b", b=B)[:, :, b]
                kts = []
                for c in range(nch):
                    Lv = min(128, L - c * 128)
                    ktp = ps_kt.tile([128, 128], F32, tag="ps_kt")
                    nc.tensor.transpose(
                        ktp[:, 0:Lv], KVt[0:Lv, c * CW:c * CW + 128],
                        ident[0:Lv, 0:Lv])
                    kt = ktpool.tile([128, 128], F32, tag="KT")
                    nc.vector.tensor_copy(kt[:, 0:Lv], ktp[:, 0:Lv])
                    kts.append(kt)
                for c in range(nch):
                    Lv = min(128, L - c * 128)
                    nc.tensor.matmul(qk[0:Lv, c * G:(c + 1) * G],
                                     kts[c][:, 0:Lv], rq,
                                     start=True, stop=True)
                nc.scalar.activation(sc[:, 0:nch * G], qk[:, 0:nch * G],
                                     mybir.ActivationFunctionType.Exp,
                                     scale=SCALE)
                pv = ps_pv.tile([G, 129], F32, tag="ps_pv")
                for c in range(nch):
                    Lv = min(128, L - c * 128)
                    nc.tensor.matmul(pv[:], sc[0:Lv, c * G:(c + 1) * G],
                                     KVt[0:Lv, c * CW + 128:(c + 1) * CW],
                                     start=(c == 0), stop=(c == nch - 1))
                # normalize by the ones-column denominator, then DMA the rows
                # into the batched [b*4+g, d] layout (partition remap)
                rcp = nrmpool.tile([G, 1], F32, tag="rcp")
                att = nrmpool.tile([G, HEAD_DIM], F32, tag="att")
                nc.vector.reciprocal(rcp[:], pv[:, 128:129])
                nc.vector.tensor_scalar_mul(att[:], pv[:, 0:128], rcp[:])
                nc.gpsimd.dma_start(pvn[G * b:G * (b + 1), :], att[:])

        # ---- phase 3: transpose attention output + projection ------------
        with ExitStack() as s4:
            ps_t = s4.enter_context(
                tc.tile_pool(name="ps_t", bufs=1, space="PSUM"))
            pvt_ps = ps_t.tile([128, 128], F32, tag="ps_t")
            nc.tensor.transpose(pvt_ps[:], pvn[:], ident[:])
            nc.vector.tensor_copy(pvT[:], pvt_ps[:])

        with ExitStack() as s5:
            wop = s5.enter_context(tc.tile_pool(name="wo", bufs=2))
            outp = s5.enter_context(tc.tile_pool(name="outp", bufs=1))
            ps_o = s5.enter_context(
                tc.tile_pool(name="ps_o", bufs=8, space="PSUM"))
            out_sb = outp.tile([B, D], F32, tag="out")
            o_ps = [ps_o.tile([B, 512], F32, tag="ps_o", name=f"ops{n}")
                    for n in range(8)]
            pvr = pvT[:].rearrange("p (b g) -> p b g", g=G)
            for g in range(G):
                wo_t = wop.tile([128, D], F32, tag="wo")
                eng = nc.sync if g % 2 == 0 else nc.scalar
                eng.dma_start(wo_t[:], wo_d[g * 128:(g + 1) * 128, :])
                lt = pvr[:, :, g]
                for n in range(8):
                    nc.tensor.matmul(o_ps[n][:], lt,
                                     wo_t[:, n * 512:(n + 1) * 512],
                                     start=(g == 0), stop=(g == G - 1))
            for n in range(8):
                nc.vector.tensor_copy(out_sb[:, n * 512:(n + 1) * 512],
                                      o_ps[n][:])
            nc.sync.dma_start(out_d[:], out_sb[:])

    nc.compile()
    return nc


def kernel(x, Wq, Wk, Wv, Wo, key_cache, value_cache, block_tables,
           context_lens):
    global LAST_RESULTS
    x = np.asarray(x, dtype=np.float32).reshape(B, D)
    xT = np.ascontiguousarray(x.T)
    Wq = np.asarray(Wq, dtype=np.float32)
    Wk = np.asarray(Wk, dtype=np.float32)
    Wv = np.asarray(Wv, dtype=np.float32)
    Wo = np.asarray(Wo, dtype=np.float32)
    key_cache = np.asarray(key_cache, dtype=np.float32)
    value_cache = np.asarray(value_cache, dtype=np.float32)
    bt = np.asarray(block_tables, dtype=np.int64)
    cl = np.asarray(context_lens, dtype=np.int64)

    Ls = [int(v) for v in cl]
    pos = np.array([v - 1 for v in Ls], dtype=np.int64)

    # rope tables at the new token's position (f32 like the reference)
    half = HEAD_DIM // 2
    inv_freq = (1.0 / (ROPE_BASE ** (np.arange(half, dtype=np.float32) / half))
                ).astype(np.float32)
    ang = pos.astype(np.float32)[:, None] * inv_freq[None, :]
    cb = np.cos(ang).astype(np.float32)          # [B, 64]
    sb = np.sin(ang).astype(np.float32)
    cq = np.ascontiguousarray(cb.T)              # [64, B]
    sq = np.ascontiguousarray(sb.T)
    ident = np.eye(128, dtype=np.float32)

    runs = [_kv_runs(bt[b], Ls[b]) for b in range(B)]

    nc = _build_nc(Ls, runs)

    in_maps = []
    for h in range(N_CORES):
        kv = np.concatenate(
            [key_cache[:, h, :], value_cache[:, h, :]], axis=1)
        in_maps.append({
            "xT": xT,
            "wq": np.ascontiguousarray(Wq[:, h * GD:(h + 1) * GD]),
            "wk": np.ascontiguousarray(Wk[:, h * HEAD_DIM:(h + 1) * HEAD_DIM]),
            "wv": np.ascontiguousarray(Wv[:, h * HEAD_DIM:(h + 1) * HEAD_DIM]),
            "wo": np.ascontiguousarray(Wo[h * GD:(h + 1) * GD, :]),
            "kv": kv,
            "cq": cq, "sq": sq, "cb": cb, "sb": sb, "ident": ident,
        })

    res = run_bass_kernel_spmd(nc, in_maps, list(range(N_CORES)))
    LAST_RESULTS = res

    out = np.zeros((B, D), dtype=np.float32)
    for h in range(N_CORES):
        out += res.results[h]["out"]
    return np.ascontiguousarray(out.reshape(B, 1, D))


# revision 14
# speedup vs baseline: 1.3184x; 1.3184x over previous
"""Paged-attention decode kernel for Trainium2 (Bass/Tile), 8 NeuronCores.

Sharding: one KV head per core (N_KV=8). Each core gets x^T plus its head's
slices of Wq/Wk/Wv/Wo and of the paged K/V caches (K|V rows interleaved into
one [slots, 256] tensor so each gathered row is a single contiguous 1KB DMA
piece), computes its 4 query heads' attention and a partial output projection
[B, D]; the host sums the 8 partials.

Only the valid context rows (t < context_lens[b]) are streamed from the cache;
the program is specialized at trace time to the actual context_lens /
block_tables (both are needed host-side anyway to build gather patterns).
"""
import os
import sys
from contextlib import ExitStack

import numpy as np

for _p in ("/opt/trn_rl_repo", "/opt/pypackages"):
    if os.path.isdir(_p) and _p not in sys.path:
        sys.path.append(_p)

import concourse.bass as bass  # noqa: E402
import concourse.tile as tile  # noqa: E402
from concourse import bacc, mybir  # noqa: E402
from concourse.bass_utils import run_bass_kernel_spmd  # noqa: E402

N_HEADS = 32
N_KV = 8
HEAD_DIM = 128
BLOCK_SIZE = 16
MAX_SEQ = 2048
ROPE_BASE = 10000.0
SCALE = HEAD_DIM ** -0.5
B = 32
D = 4096
G = N_HEADS // N_KV  # 4 query heads per kv head
GD = G * HEAD_DIM    # 512
N_CORES = 8
MAX_CH = MAX_SEQ // 128  # 16
KVW = 2 * HEAD_DIM       # 256: one K row + one V row
CW = KVW + 2             # 258: chunk block width incl. ones column
                         # (+pad so the PV moving dim is even, an fp32r
                         # ISA requirement)

F32 = mybir.dt.float32
F32R = mybir.dt.float32r

LAST_RESULTS = None  # test harness reads exec_time_ns from here


def _kv_runs(bt_row, L):
    """Maximal runs of consecutive cache rows covering t in [0, L).

    Returns [(row_start, t_start, n_rows)].  With an arange block table this
    is a single run."""
    nblocks = (L + BLOCK_SIZE - 1) // BLOCK_SIZE
    runs = []
    cur_s = cur_t = cur_n = 0
    for j in range(nblocks):
        rows = min(BLOCK_SIZE, L - j * BLOCK_SIZE)
        s = int(bt_row[j]) * BLOCK_SIZE
        if cur_n and s == cur_s + cur_n:
            cur_n += rows
        else:
            if cur_n:
                runs.append((cur_s, cur_t, cur_n))
            cur_s, cur_t, cur_n = s, j * BLOCK_SIZE, rows
    if cur_n:
        runs.append((cur_s, cur_t, cur_n))
    return runs


def _emit_kv_dmas(engs, kv_d, tl, runs):
    """DMA interleaved K|V cache rows into tile `tl` laid out
    [128 (t%128), (t//128)*257 + d] with col 256 of each block left for the
    ones column.  One dma_start per 128-row chunk keeps each transfer's HBM
    reads sequential (partition p <-> row p within the chunk)."""
    ei = 0
    for (row0, t0, n) in runs:
        while n > 0:
            c = t0 // 128
            p0 = t0 % 128
            take = min(n, 128 - p0)
            src = kv_d[row0:row0 + take, :]
            dst = tl[p0:p0 + take, c * CW:c * CW + KVW]
            engs[ei % len(engs)].dma_start(dst, src)
            ei += 1
            row0 += take
            t0 += take
            n -= take


def _mmr(nc, out, lhsT, rhs, **kw):
    # float32r: single-pass PE matmul (fp32 lowers to two HI/LO passes)
    nc.tensor.matmul(out, lhsT.bitcast(F32R), rhs.bitcast(F32R), **kw)


def _build_nc(Ls, runs_all):
    nc = bacc.Bacc("TRN2", target_bir_lowering=False, debug=False,
                   num_devices=N_CORES)

    xt_d = nc.declare_dram_parameter("xT", [D, B], F32R, isOutput=False)
    wq_d = nc.declare_dram_parameter("wq", [D, GD], F32R, isOutput=False)
    wk_d = nc.declare_dram_parameter("wk", [D, HEAD_DIM], F32R, isOutput=False)
    wv_d = nc.declare_dram_parameter("wv", [D, HEAD_DIM], F32R, isOutput=False)
    wo_d = nc.declare_dram_parameter("wo", [GD, D], F32R, isOutput=False)
    kv_d = nc.declare_dram_parameter("kv", [B * MAX_SEQ, KVW], F32R,
                                     isOutput=False)
    cq_d = nc.declare_dram_parameter("cq", [64, B], F32, isOutput=False)
    sq_d = nc.declare_dram_parameter("sq", [64, B], F32, isOutput=False)
    cb_d = nc.declare_dram_parameter("cb", [B, 64], F32, isOutput=False)
    sb_d = nc.declare_dram_parameter("sb", [B, 64], F32, isOutput=False)
    id_d = nc.declare_dram_parameter("ident", [128, 128], F32, isOutput=False)
    out_d = nc.declare_dram_parameter("out", [B, D], F32, isOutput=True)

    with tile.TileContext(nc) as tc, ExitStack() as top:
        cpool = top.enter_context(tc.tile_pool(name="const", bufs=1))
        ident = cpool.tile([128, 128], F32, tag="ident")
        nc.sync.dma_start(ident[:], id_d[:])
        qT = cpool.tile([128, G * B], F32R, tag="qT")      # [d, g*32+b] (roped)
        knvn = cpool.tile([B, KVW], F32R, tag="knvn")      # roped new k | new v
        pvn = cpool.tile([128, 128], F32, tag="pvn")      # normalized [b*4+g, d]
        pvT = cpool.tile([128, 128], F32R, tag="pvT")      # [d, b*4+g]

        # ---- phase 1: q/k/v projections + rope ---------------------------
        with ExitStack() as s1:
            p1 = s1.enter_context(tc.tile_pool(name="p1", bufs=1))
            wqp = s1.enter_context(tc.tile_pool(name="wqp", bufs=4))
            ps_q = s1.enter_context(
                tc.tile_pool(name="ps_q", bufs=4, space="PSUM"))
            ps_kv = s1.enter_context(
                tc.tile_pool(name="ps_kv", bufs=2, space="PSUM"))
            tmp = s1.enter_context(tc.tile_pool(name="rtmp", bufs=4))

            xT = p1.tile([128, 32 * B], F32R, tag="xT")    # [d, kc*32+b]
            nc.sync.dma_start(
                xT[:].rearrange("p (kc b) -> p kc b", b=B),
                xt_d[:].rearrange("(kc p) b -> p kc b", p=128))
            cq = p1.tile([64, B], F32, tag="cq")
            sq = p1.tile([64, B], F32, tag="sq")
            cb = p1.tile([B, 64], F32, tag="cb")
            sb = p1.tile([B, 64], F32, tag="sb")
            nc.sync.dma_start(cq[:], cq_d[:])
            nc.sync.dma_start(sq[:], sq_d[:])
            nc.sync.dma_start(cb[:], cb_d[:])
            nc.sync.dma_start(sb[:], sb_d[:])
            wk_sb = p1.tile([128, 32 * HEAD_DIM], F32R, tag="wk")
            wv_sb = p1.tile([128, 32 * HEAD_DIM], F32R, tag="wv")
            nc.scalar.dma_start(
                wk_sb[:].rearrange("p (kc m) -> p kc m", m=HEAD_DIM),
                wk_d[:].rearrange("(kc p) m -> p kc m", p=128))
            nc.scalar.dma_start(
                wv_sb[:].rearrange("p (kc m) -> p kc m", m=HEAD_DIM),
                wv_d[:].rearrange("(kc p) m -> p kc m", p=128))

            qg_ps = [ps_q.tile([128, B], F32, tag="ps_q", name=f"qg{g}")
                     for g in range(G)]
            k_ps = ps_kv.tile([B, HEAD_DIM], F32, tag="ps_kv")
            v_ps = ps_kv.tile([B, HEAD_DIM], F32, tag="ps_kv")
            for kc in range(32):
                wq_t = wqp.tile([128, GD], F32R, tag="wq")
                eng = nc.sync if kc % 2 == 0 else nc.scalar
                eng.dma_start(wq_t[:], wq_d[kc * 128:(kc + 1) * 128, :])
                rx = xT[:, kc * B:(kc + 1) * B]
                for g in range(G):
                    _mmr(nc, qg_ps[g][:], wq_t[:, g * 128:(g + 1) * 128], rx,
                         start=(kc == 0), stop=(kc == 31))
                _mmr(nc, k_ps[:], rx, wk_sb[:, kc * 128:(kc + 1) * 128],
                     start=(kc == 0), stop=(kc == 31))
                _mmr(nc, v_ps[:], rx, wv_sb[:, kc * 128:(kc + 1) * 128],
                     start=(kc == 0), stop=(kc == 31))

            # rope q: rows = d within head, cols = b; per g
            for g in range(G):
                q0 = qg_ps[g][0:64, :]
                q1 = qg_ps[g][64:128, :]
                o0 = qT[0:64, g * B:(g + 1) * B]
                o1 = qT[64:128, g * B:(g + 1) * B]
                t1 = tmp.tile([64, B], F32, tag="rt1")
                t2 = tmp.tile([64, B], F32, tag="rt2")
                nc.vector.tensor_mul(t1[:], q0, cq[:])
                nc.vector.tensor_mul(t2[:], q1, sq[:])
                nc.vector.tensor_sub(o0, t1[:], t2[:])
                t3 = tmp.tile([64, B], F32, tag="rt1")
                t4 = tmp.tile([64, B], F32, tag="rt2")
                nc.vector.tensor_mul(t3[:], q0, sq[:])
                nc.vector.tensor_mul(t4[:], q1, cq[:])
                nc.vector.tensor_add(o1, t3[:], t4[:])

            # rope k (natural layout [b, d]) into knvn; v straight copy
            k0 = k_ps[:, 0:64]
            k1 = k_ps[:, 64:128]
            u1 = tmp.tile([B, 64], F32, tag="ru1")
            u2 = tmp.tile([B, 64], F32, tag="ru2")
            nc.vector.tensor_mul(u1[:], k0, cb[:])
            nc.vector.tensor_mul(u2[:], k1, sb[:])
            nc.vector.tensor_sub(knvn[:, 0:64], u1[:], u2[:])
            u3 = tmp.tile([B, 64], F32, tag="ru1")
            u4 = tmp.tile([B, 64], F32, tag="ru2")
            nc.vector.tensor_mul(u3[:], k0, sb[:])
            nc.vector.tensor_mul(u4[:], k1, cb[:])
            nc.vector.tensor_add(knvn[:, 64:128], u3[:], u4[:])
            nc.vector.tensor_copy(knvn[:, 128:256], v_ps[:])

        # ---- phase 2: per-request attention ------------------------------
        with ExitStack() as s3:
            kvpool = s3.enter_context(tc.tile_pool(name="KV", bufs=3))
            ktpool = s3.enter_context(tc.tile_pool(name="KT", bufs=20))
            scpool = s3.enter_context(tc.tile_pool(name="SC", bufs=3))
            ps_kt = s3.enter_context(
                tc.tile_pool(name="ps_kt", bufs=3, space="PSUM"))
            ps_qk = s3.enter_context(
                tc.tile_pool(name="ps_qk", bufs=2, space="PSUM"))
            ps_pv = s3.enter_context(
                tc.tile_pool(name="ps_pv", bufs=2, space="PSUM"))
            nrmpool = s3.enter_context(tc.tile_pool(name="nrm", bufs=3))

            for b in range(B):
                L = Ls[b]
                pos = L - 1
                nch = (L + 127) // 128
                KVt = kvpool.tile([128, MAX_CH * CW], F32R, tag="KV")
                _emit_kv_dmas([nc.sync, nc.scalar], kv_d, KVt, runs_all[b])
                # overwrite the new token's row (DMA: partition remap)
                prow, pch = pos % 128, pos // 128
                nc.gpsimd.dma_start(
                    KVt[prow:prow + 1, pch * CW:pch * CW + KVW],
                    knvn[b:b + 1, :])
                # ones column for the softmax denominator
                ones_ap = KVt[:].rearrange("p (ch e) -> p ch e", e=CW)[
                    :, 0:nch, KVW:KVW + 2]
                nc.gpsimd.memset(ones_ap.bitcast(F32), 1.0)

                qk = ps_qk.tile([128, MAX_CH * G], F32, tag="ps_qk")
                sc = scpool.tile([128, MAX_CH * G], F32R, tag="SC")
                rq = qT[:].rearrange("p (g b) -> p g b", b=B)[:, :, b]
                kts = []
                for c in range(nch):
                    Lv = min(128, L - c * 128)
                    ktp = ps_kt.tile([128, 128], F32, tag="ps_kt")
                    nc.tensor.transpose(
                        ktp[:, 0:Lv],
                        KVt[0:Lv, c * CW:c * CW + 128].bitcast(F32),
                        ident[0:Lv, 0:Lv])
                    kt = ktpool.tile([128, 128], F32R, tag="KT")
                    nc.vector.tensor_copy(kt[:, 0:Lv], ktp[:, 0:Lv])
                    kts.append(kt)
                for c in range(nch):
                    Lv = min(128, L - c * 128)
                    _mmr(nc, qk[0:Lv, c * G:(c + 1) * G],
                         kts[c][:, 0:Lv], rq, start=True, stop=True)
                nc.scalar.activation(sc[:, 0:nch * G], qk[:, 0:nch * G],
                                     mybir.ActivationFunctionType.Exp,
                                     scale=SCALE)
                pv = ps_pv.tile([G, 130], F32, tag="ps_pv")
                for c in range(nch):
                    Lv = min(128, L - c * 128)
                    _mmr(nc, pv[:], sc[0:Lv, c * G:(c + 1) * G],
                         KVt[0:Lv, c * CW + 128:(c + 1) * CW],
                         start=(c == 0), stop=(c == nch - 1))
                # normalize by the ones-column denominator, then DMA the rows
                # into the batched [b*4+g, d] layout (partition remap)
                rcp = nrmpool.tile([G, 1], F32, tag="rcp")
                att = nrmpool.tile([G, HEAD_DIM], F32, tag="att")
                nc.vector.reciprocal(rcp[:], pv[:, 128:129])
                nc.vector.tensor_scalar_mul(att[:], pv[:, 0:128], rcp[:])
                nc.gpsimd.dma_start(pvn[G * b:G * (b + 1), :], att[:])

        # ---- phase 3: transpose attention output + projection ------------
        with ExitStack() as s4:
            ps_t = s4.enter_context(
                tc.tile_pool(name="ps_t", bufs=1, space="PSUM"))
            pvt_ps = ps_t.tile([128, 128], F32, tag="ps_t")
            nc.tensor.transpose(pvt_ps[:], pvn[:], ident[:])
            nc.vector.tensor_copy(pvT[:], pvt_ps[:])

        with ExitStack() as s5:
            wop = s5.enter_context(tc.tile_pool(name="wo", bufs=2))
            outp = s5.enter_context(tc.tile_pool(name="outp", bufs=1))
            ps_o = s5.enter_context(
                tc.tile_pool(name="ps_o", bufs=8, space="PSUM"))
            out_sb = outp.tile([B, D], F32, tag="out")
            o_ps = [ps_o.tile([B, 512], F32, tag="ps_o", name=f"ops{n}")
                    for n in range(8)]
            pvr = pvT[:].rearrange("p (b g) -> p b g", g=G)
            for g in range(G):
                wo_t = wop.tile([128, D], F32R, tag="wo")
                eng = nc.sync if g % 2 == 0 else nc.scalar
                eng.dma_start(wo_t[:], wo_d[g * 128:(g + 1) * 128, :])
                lt = pvr[:, :, g]
                for n in range(8):
                    _mmr(nc, o_ps[n][:], lt, wo_t[:, n * 512:(n + 1) * 512],
                         start=(g == 0), stop=(g == G - 1))
            for n in range(8):
                nc.vector.tensor_copy(out_sb[:, n * 512:(n + 1) * 512],
                                      o_ps[n][:])
            nc.sync.dma_start(out_d[:], out_sb[:])

    nc.compile()
    return nc


def kernel(x, Wq, Wk, Wv, Wo, key_cache, value_cache, block_tables,
           context_lens):
    global LAST_RESULTS
    x = np.asarray(x, dtype=np.float32).reshape(B, D)
    xT = np.ascontiguousarray(x.T)
    Wq = np.asarray(Wq, dtype=np.float32)
    Wk = np.asarray(Wk, dtype=np.float32)
    Wv = np.asarray(Wv, dtype=np.float32)
    Wo = np.asarray(Wo, dtype=np.float32)
    key_cache = np.asarray(key_cache, dtype=np.float32)
    value_cache = np.asarray(value_cache, dtype=np.float32)
    bt = np.asarray(block_tables, dtype=np.int64)
    cl = np.asarray(context_lens, dtype=np.int64)

    Ls = [int(v) for v in cl]
    pos = np.array([v - 1 for v in Ls], dtype=np.int64)

    # rope tables at the new token's position (f32 like the reference)
    half = HEAD_DIM // 2
    inv_freq = (1.0 / (ROPE_BASE ** (np.arange(half, dtype=np.float32) / half))
                ).astype(np.float32)
    ang = pos.astype(np.float32)[:, None] * inv_freq[None, :]
    cb = np.cos(ang).astype(np.float32)          # [B, 64]
    sb = np.sin(ang).astype(np.float32)
    cq = np.ascontiguousarray(cb.T)              # [64, B]
    sq = np.ascontiguousarray(sb.T)
    ident = np.eye(128, dtype=np.float32)

    runs = [_kv_runs(bt[b], Ls[b]) for b in range(B)]

    nc = _build_nc(Ls, runs)

    in_maps = []
    for h in range(N_CORES):
        kv = np.concatenate(
            [key_cache[:, h, :], value_cache[:, h, :]], axis=1)
        in_maps.append({
            "xT": xT,
            "wq": np.ascontiguousarray(Wq[:, h * GD:(h + 1) * GD]),
            "wk": np.ascontiguousarray(Wk[:, h * HEAD_DIM:(h + 1) * HEAD_DIM]),
            "wv": np.ascontiguousarray(Wv[:, h * HEAD_DIM:(h + 1) * HEAD_DIM]),
            "wo": np.ascontiguousarray(Wo[h * GD:(h + 1) * GD, :]),
            "kv": kv,
            "cq": cq, "sq": sq, "cb": cb, "sb": sb, "ident": ident,
        })

    res = run_bass_kernel_spmd(nc, in_maps, list(range(N_CORES)))
    LAST_RESULTS = res

    out = np.zeros((B, D), dtype=np.float32)
    for h in range(N_CORES):
        out += res.results[h]["out"]
    return np.ascontiguousarray(out.reshape(B, 1, D))
